# revision 1
# baseline (speedup 1.0000x reference)
"""Trainium2 Bass kernel for Chronos2Attention (B=4, S=2048, D=1024, H=16, Dh=64).

Sharding: 8 cores = 4 batches x 2 head-groups. Core c handles batch c//2 and
heads 8*(c%2) .. 8*(c%2)+7 (tensor-parallel over heads: wq/wk/wv column-sharded,
wo row-sharded a la vLLM QKVParallelLinear/RowParallelLinear). Each core emits a
partial [S, D] output for its batch; the host sums the two partials per batch
(the RowParallelLinear all-reduce, done at gather time).

Per-core device pipeline:
  1. Q/K/V projections in natural [S, 512] layout (PE, K=1024 contraction),
     with hidden^T pre-tiled on the host so it is directly usable as lhsT.
  2. RoPE applied on DVE in natural layout (free-dim shifts with a signed sin
     table); V copied into a [S, 8*65] tile with a ones column appended per
     head (the ones column makes the attention row-sums fall out of the AV
     matmul for free).
  3. PE transposes Qrot/Krot -> QT/KT [512, S] (head_dim on partitions).
  4. Per (query-block j, head h): scores^T = K_h @ Q_h^T on PE (contraction
     over head_dim=64), exp on ACT (no max subtraction; scores are O(30) so
     fp32 exp is safe), AV'^T accumulated over key chunks on PE. Row 64 of
     AV'^T is the softmax denominator; reciprocal (DVE) is broadcast across
     the 64 head partitions with a tiny K=1 PE matmul, and the normalization
     multiply lands in a per-block attn^T SBUF tile.
  5. Output projection out = attn^T.T @ wo on PE per query block, DMA out.
"""

import numpy as np

import concourse.bacc as bacc
import concourse.mybir as mybir
import concourse.tile as tile
from concourse import bass_utils

# Problem shapes (hardcoded per spec)
B = 4
S = 2048
D = 1024
H = 16
DH = 64
ROPE_THETA = 10000.0
NCORES = 8
HC = H // 2  # heads per core
MC = HC * DH  # 512, per-core projection width

SM = S // 128  # 16 seq chunks
KD = D // 128  # 8 contraction chunks for projections
MD = MC // 128  # 4 head-dim chunks per core
JBLK = 512  # query-block size for attention
NJ = S // JBLK  # 4

F32 = mybir.dt.float32
# float32r: relaxed fp32 matmul, 4x faster than float32 on the PE at N>=256.
# The BIR verifier requires every producer of an fp32r-matmul operand to emit
# float32r (rounded), so matmul-feeding tensors/tiles are declared MF.
USE_F32R = True
MF = mybir.dt.float32r if USE_F32R else F32


def build_nc():
    nc = bacc.Bacc("TRN2", target_bir_lowering=False, debug=False, num_devices=1)

    hT = nc.dram_tensor("hT", [SM, 128, D], MF, kind="ExternalInput").ap()
    wq = nc.dram_tensor("wq", [128, KD * MC], MF, kind="ExternalInput").ap()
    wk = nc.dram_tensor("wk", [128, KD * MC], MF, kind="ExternalInput").ap()
    wv = nc.dram_tensor("wv", [128, KD * MC], MF, kind="ExternalInput").ap()
    wo = nc.dram_tensor("wo", [128, MD * D], MF, kind="ExternalInput").ap()
    cosq = nc.dram_tensor("cosq", [128, SM * DH], F32, kind="ExternalInput").ap()
    sinq = nc.dram_tensor("sinq", [128, SM * DH], F32, kind="ExternalInput").ap()
    ident = nc.dram_tensor("ident", [128, 128], MF, kind="ExternalInput").ap()
    onesd = nc.dram_tensor("onesd", [128, 128], MF, kind="ExternalInput").ap()
    out = nc.dram_tensor("out", [S, D], F32, kind="ExternalOutput").ap()

    with tile.TileContext(nc) as tc:
        _build_body(nc, tc, hT, wq, wk, wv, wo, cosq, sinq, ident, onesd, out)
    nc.compile()
    return nc


def _build_body(nc, tc, hT, wq, wk, wv, wo, cosq, sinq, ident, onesd, out):
    from contextlib import ExitStack

    Exp = mybir.ActivationFunctionType.Exp

    with ExitStack() as ctx:
        # ---- persistent tiles (live through attention) ----
        persist = ctx.enter_context(tc.tile_pool(name="persist", bufs=1))
        qt = [persist.tile([128, S], MF, tag=f"qt{d}", name=f"qt{d}") for d in range(MD)]
        kt = [persist.tile([128, S], MF, tag=f"kt{d}", name=f"kt{d}") for d in range(MD)]
        v1 = persist.tile([128, SM * (HC * 65)], MF, tag="v1", name="v1")  # [128, 8320]
        ident_t = persist.tile([128, 128], MF, tag="ident", name="ident_t")
        ones_t = persist.tile([128, 128], MF, tag="ones", name="ones_t")
        ones_col = ones_t[0:1, 0:DH]

        nc.sync.dma_start(out=ident_t[:], in_=ident)
        nc.sync.dma_start(out=ones_t[:], in_=onesd)
        # touch Exp early so the ACT table DMA (~2.7us) overlaps phase A
        warm_ex = persist.tile([1, 16], F32, tag="warm_ex", name="warm_ex")
        nc.scalar.activation(warm_ex[:], ones_t[0:1, 0:16].bitcast(F32), Exp)
        # ones columns of v1: position m*520 + h*65 + 64
        nc.vector.tensor_copy(
            v1[:].rearrange("p (m h e) -> p m h e", m=SM, h=HC)[:, :, :, 64:65],
            ones_t[:, None, None, 0:1].broadcast_to([128, SM, HC, 1]),
        )

        # ---- phase A: projections + rope + transposes ----
        with ExitStack() as actx:
            consts = actx.enter_context(tc.tile_pool(name="constsA", bufs=1))
            cos_t = consts.tile([128, SM * DH], F32, tag="cos", name="cos_t")  # [128, 1024]
            sin_t = consts.tile([128, SM * DH], F32, tag="sin", name="sin_t")
            w_t = {
                n: consts.tile([128, KD * MC], MF, tag=f"w_{n}", name=f"w_{n}")  # [128, 4096]
                for n in ("q", "k", "v")
            }
            # per-chunk weight DMAs so the first projection matmuls only
            # wait on 2KB/partition, not the whole 16KB weight tile
            for n, w in (("q", wq), ("k", wk), ("v", wv)):
                for kk in range(KD):
                    nc.sync.dma_start(
                        out=w_t[n][:, kk * MC : (kk + 1) * MC],
                        in_=w[:, kk * MC : (kk + 1) * MC],
                    )
                if n == "q":
                    nc.sync.dma_start(out=cos_t[:], in_=cosq)
                    nc.sync.dma_start(out=sin_t[:], in_=sinq)

            hpool = actx.enter_context(tc.tile_pool(name="hT", bufs=2))
            qrot = actx.enter_context(tc.tile_pool(name="qrot", bufs=5))
            tmp = actx.enter_context(tc.tile_pool(name="ropetmp", bufs=2))
            proj_ps = actx.enter_context(
                tc.tile_pool(name="proj_ps", bufs=3, space="PSUM")
            )
            tp_ps = actx.enter_context(tc.tile_pool(name="tp_ps", bufs=2, space="PSUM"))

            TG = 4  # transpose group size (m chunks per PE-transpose batch)
            rot = {"q": [], "k": []}  # rot tiles pending transpose
            for m in range(SM):
                h_m = hpool.tile([128, D], MF, tag="h", name="h_m")
                nc.sync.dma_start(out=h_m[:], in_=hT[m])
                for n in ("q", "k", "v"):
                    ps = proj_ps.tile([128, MC], F32, tag="proj", name="ps")
                    for kk in range(KD):
                        nc.tensor.matmul(
                            ps[:],
                            (h_m[:, kk * 128 : (kk + 1) * 128]),
                            (w_t[n][:, kk * MC : (kk + 1) * MC]),
                            start=(kk == 0),
                            stop=(kk == KD - 1),
                        )
                    if n == "v":
                        # strided copy into v1 (65-wide per head, ones kept)
                        dst = v1[:, m * (HC * 65) : (m + 1) * (HC * 65)].rearrange(
                            "p (h e) -> p h e", h=HC
                        )[:, :, 0:64]
                        nc.scalar.copy(
                            dst, ps[:].rearrange("p (h e) -> p h e", h=HC)
                        )
                    else:
                        # RoPE: r = p*cos + shift(p)*sin_signed
                        cos_m = cos_t[:, None, m * DH : (m + 1) * DH]
                        sin_m = sin_t[:, m * DH : (m + 1) * DH]
                        r = qrot.tile([128, MC], MF, tag=f"r{n}", name=f"r{n}")
                        tc_ = tmp.tile([128, MC], F32, tag="tc", name="tc_")
                        ts_ = tmp.tile([128, MC], F32, tag="ts", name="ts_")
                        p3 = ps[:].rearrange("p (h e) -> p h e", h=HC)
                        t3 = ts_[:].rearrange("p (h e) -> p h e", h=HC)
                        nc.vector.tensor_mul(
                            tc_[:].rearrange("p (h e) -> p h e", h=HC),
                            p3,
                            cos_m.broadcast_to([128, HC, DH]),
                        )
                        nc.vector.tensor_mul(
                            t3[:, :, 0:32],
                            p3[:, :, 32:64],
                            sin_m[:, None, 0:32].broadcast_to([128, HC, 32]),
                        )
                        nc.vector.tensor_mul(
                            t3[:, :, 32:64],
                            p3[:, :, 0:32],
                            sin_m[:, None, 32:64].broadcast_to([128, HC, 32]),
                        )
                        nc.vector.tensor_add(r[:], tc_[:], ts_[:])
                        rot[n].append(r)

                # transpose every TG m-chunks: rot[n][...] -> qt/kt columns
                if m % TG == TG - 1:
                    m0 = m - (TG - 1)
                    for n, dstt in (("q", qt), ("k", kt)):
                        for d in range(MD):
                            tps = tp_ps.tile([128, TG * 128], MF, tag="tp", name="tps")
                            for mm in range(TG):
                                nc.tensor.transpose(
                                    tps[:, mm * 128 : (mm + 1) * 128],
                                    rot[n][mm][:, d * 128 : (d + 1) * 128],
                                    ident_t[:],
                                )
                            nc.scalar.copy(
                                dstt[d][:, m0 * 128 : m0 * 128 + TG * 128], tps[:]
                            )
                    rot = {"q": [], "k": []}

        # ---- phase B: attention + output projection per query block ----
        with ExitStack() as bctx:
            constsB = bctx.enter_context(tc.tile_pool(name="constsB", bufs=1))
            wo_t = constsB.tile([128, MD * D], MF, tag="wo", name="wo_t")  # [128, 4096]
            nc.sync.dma_start(out=wo_t[:], in_=wo)
            expp = bctx.enter_context(tc.tile_pool(name="expp", bufs=4))
            rcpp = bctx.enter_context(tc.tile_pool(name="rcpp", bufs=2))
            stgp = bctx.enter_context(tc.tile_pool(name="stgp", bufs=3))
            attp = bctx.enter_context(tc.tile_pool(name="attp", bufs=2))
            outp = bctx.enter_context(tc.tile_pool(name="outp", bufs=2))
            aps = bctx.enter_context(tc.tile_pool(name="aps", bufs=1, space="PSUM"))

            wo_state = {}

            def emit_wo_piece(jw, att_w, mq, nb):
                # half of one 128-row chunk of block jw's output projection:
                # 4 accumulating matmuls + immediate psum eviction, small
                # enough to hide between two exp-paced m iterations
                mrow = jw * JBLK + mq * 128
                wops = aps.tile([128, 512], F32, tag="sc", name="wops", bufs=2)
                for kk in range(MD):
                    nc.tensor.matmul(
                        wops[:],
                        att_w[kk][:, mq * 128 : (mq + 1) * 128],
                        wo_t[:, kk * D + nb * 512 : kk * D + nb * 512 + 512],
                        start=(kk == 0),
                        stop=(kk == MD - 1),
                    )
                if nb == 0:
                    wo_state["ot"] = outp.tile([128, D], F32, tag="ot", name="ot")
                nc.vector.tensor_copy(
                    wo_state["ot"][:, nb * 512 : (nb + 1) * 512], wops[:]
                )
                if nb == 1:
                    nc.sync.dma_start(
                        out=out[mrow : mrow + 128, :], in_=wo_state["ot"][:]
                    )

            def emit_wo_chunk(jw, att_w, mq):
                emit_wo_piece(jw, att_w, mq, 0)
                emit_wo_piece(jw, att_w, mq, 1)

            # dep-free dense K=128 burst to re-warm the PE clock after the
            # phase handoff gap (HAM re-throttles on any >3.4us idle)
            wub = aps.tile([128, 2 * JBLK], F32, tag="sc", name="wub", bufs=2)
            for i in range(12):
                nc.tensor.matmul(
                    wub[:, 0:512],
                    wo_t[:, 0:128],
                    wo_t[:, 1024 : 1024 + 512],
                    start=(i == 0),
                    stop=(i == 11),
                )

            prev_wo = None
            for j in range(NJ):
                jc = j * JBLK
                att = [attp.tile([128, JBLK], MF, tag=f"att{d}", name=f"att{d}") for d in range(MD)]
                for d in range(MD):
                    # two heads (rows 0-63 / 64-127 of qt/kt chunk d) packed
                    # into concurrent row-group matmuls to fill the PE array;
                    # one [128, 2*JBLK] psum tile so exp is a single ACT op
                    hA, hB = 2 * d, 2 * d + 1
                    avA = aps.tile([65, JBLK], F32, tag="avA", name="avA", bufs=2)
                    avB = aps.tile([65, JBLK], F32, tag="avB", name="avB", bufs=2)

                    def emit_sc(m):
                        sc = aps.tile([128, 2 * JBLK], F32, tag="sc", name="sc", bufs=2)
                        nc.tensor.matmul(
                            sc[:, 0:JBLK],
                            kt[d][0:64, m * 128 : (m + 1) * 128],
                            qt[d][0:64, jc : jc + JBLK],
                            start=True,
                            stop=True,
                        )
                        nc.tensor.matmul(
                            sc[:, JBLK : 2 * JBLK],
                            kt[d][64:128, m * 128 : (m + 1) * 128],
                            qt[d][64:128, jc : jc + JBLK],
                            start=True,
                            stop=True,
                        )
                        return sc

                    # scores emitted one m ahead so the exp stream always has
                    # a ready input even when PE bursts (Wo chunks) or throttles
                    sc_next = emit_sc(0)
                    for m in range(SM):
                        sc = sc_next
                        if m + 1 < SM:
                            sc_next = emit_sc(m + 1)
                        ex = expp.tile([128, 2 * JBLK], MF, tag="ex", name="ex")
                        nc.scalar.activation(ex[:], sc[:], Exp)
                        for hh, e0, av in ((hA, 0, avA), (hB, JBLK, avB)):
                            vs = m * (HC * 65) + hh * 65
                            nc.tensor.matmul(
                                av[:],
                                v1[:, vs : vs + 65],
                                ex[:, e0 : e0 + JBLK],
                                start=(m == 0),
                                stop=(m == SM - 1),
                            )
                        if d == 1 and m % 2 == 1 and prev_wo is not None:
                            emit_wo_piece(*prev_wo, (m // 2) // 2, (m // 2) % 2)
                            if m == SM - 1:
                                prev_wo = None

                    # normalize: copy raw AV^T out (frees psum), reciprocal of
                    # the ones-column row-sum, gpsimd-broadcast it across the
                    # 64 head partitions, divide in place in SBUF
                    for p0, av in ((0, avA), (64, avB)):
                        stage = stgp.tile([64, JBLK], F32, tag="stg", name="stage")
                        nc.vector.tensor_copy(stage[:], av[0:64, :])
                        rcp = rcpp.tile([1, JBLK], F32, tag="rcp", name="rcp")
                        nc.vector.reciprocal(rcp[:], av[64:65, :])
                        rb = rcpp.tile([64, JBLK], F32, tag="rb", name="rb")
                        nc.gpsimd.partition_broadcast(rb[:], rcp[:])
                        nc.vector.tensor_mul(att[d][p0 : p0 + 64, :], stage[:], rb[:])
                prev_wo = (j, att)
            for mq in range(JBLK // 128):
                emit_wo_chunk(*prev_wo, mq)


def _wtile(w):
    """[K*128, N] -> [128, K*N] with tile[p, kk*N+c] = w[128*kk+p, c]."""
    kchunks = w.shape[0] // 128
    return np.ascontiguousarray(
        w.reshape(kchunks, 128, w.shape[1])
        .transpose(1, 0, 2)
        .reshape(128, kchunks * w.shape[1])
    )


def prep_core_inputs(positions, hidden_states, wq, wk, wv, wo):
    """Host-side sharding/pre-tiling. Returns list of 8 in_maps."""
    pos = np.asarray(positions).astype(np.float32)
    inv_freq = 1.0 / (ROPE_THETA ** (np.arange(0, DH, 2, dtype=np.float32) / DH))
    ang = pos[:, None] * inv_freq[None, :]  # [S, 32]
    cos_half = np.cos(ang).astype(np.float32)
    sin_half = np.sin(ang).astype(np.float32)
    cos_full = np.concatenate([cos_half, cos_half], axis=1)  # [S, 64]
    sin_signed = np.concatenate([-sin_half, sin_half], axis=1)  # [S, 64]
    cos_tiled = np.ascontiguousarray(
        cos_full.reshape(SM, 128, DH).transpose(1, 0, 2).reshape(128, SM * DH)
    )
    sin_tiled = np.ascontiguousarray(
        sin_signed.reshape(SM, 128, DH).transpose(1, 0, 2).reshape(128, SM * DH)
    )
    ident = np.eye(128, dtype=np.float32)
    ones = np.ones((128, 128), dtype=np.float32)

    hs = np.asarray(hidden_states, dtype=np.float32)
    # hT[m, p, kk*128+cc] = hidden[b, 128m+cc, 128kk+p]
    hT_all = []
    for b in range(B):
        t = hs[b].reshape(SM, 128, KD, 128).transpose(0, 3, 2, 1)  # (m, p, kk, cc)
        hT_all.append(np.ascontiguousarray(t.reshape(SM, 128, D)))

    in_maps = []
    for c in range(NCORES):
        b = c // 2
        g = c % 2
        sl = slice(g * MC, (g + 1) * MC)
        in_maps.append(
            {
                "hT": hT_all[b],
                "wq": _wtile(np.asarray(wq, np.float32)[:, sl]),
                "wk": _wtile(np.asarray(wk, np.float32)[:, sl]),
                "wv": _wtile(np.asarray(wv, np.float32)[:, sl]),
                "wo": _wtile(np.asarray(wo, np.float32)[sl, :]),
                "cosq": cos_tiled,
                "sinq": sin_tiled,
                "ident": ident,
                "onesd": ones,
            }
        )
    return in_maps


_NC_CACHE = {}


def get_nc():
    if "nc" not in _NC_CACHE:
        _NC_CACHE["nc"] = build_nc()
    return _NC_CACHE["nc"]


def kernel(positions, hidden_states, wq, wk, wv, wo):
    in_dtype = np.asarray(hidden_states).dtype
    in_maps = prep_core_inputs(positions, hidden_states, wq, wk, wv, wo)
    nc = get_nc()
    res = bass_utils.run_bass_kernel_spmd(nc, in_maps, core_ids=list(range(NCORES)))
    outs = np.empty((B, S, D), dtype=np.float32)
    for b in range(B):
        outs[b] = res.results[2 * b]["out"] + res.results[2 * b + 1]["out"]
    return outs.astype(in_dtype, copy=False)

